# revision 30
# baseline (speedup 1.0000x reference)
"""LSH cosine-of-Hamming retrieval kernel for 8 trn2 NeuronCores.

Math: reference computes cos((pi/d) * hamming(u, v)) for binary LSH codes
u = (emb1 @ r.T > 0), v = (emb2 @ r.T > 0), d = 1024 bits.
With +/-1 sign codes s_u = 2u-1, s_v = 2v-1:
    hamming = (d - s_u . s_v) / 2
    cos((pi/d) * hamming) = cos(pi/2 - (pi/2d) * s_u.s_v) = sin((pi/2d) * s_u.s_v)
The kernel stores half-codes c = s/2 = (x > 0) - 0.5 in fp8 (exact), so
out = sin((2*pi/d) * c_u.c_v).

Pipeline per core: bf16 hi/lo split projection matmul (3 accumulating
passes hh+hl+lh reproduce fp32 signs to ~1e-6 flip rate) -> one DVE
tensor_scalar (is_gt, subtract 0.5) -> fp8 +/-0.5 codes -> fp8 DoubleRow
code matmul (integer-exact in PSUM f32) -> Sin activation, scale pi/512.

Sharding (2x4 grid over 8 cores): core k computes the [2048, 2048] output
block for emb1 rows [(k//4)*2048...] x emb2 rows [(k%4)*2048...]; r is
replicated. This minimizes replicated projection FLOPs without
cross-core exchange (collectives measure ~60us fixed cost under this
runtime - slower than the 26us of projection work they would save).
Embeddings are passed to the device pre-transposed (dim-major) and
hi/lo-split on the host - pure data layout prep.
"""

import sys

sys.path.insert(0, "/opt/trn_rl_repo")

import ml_dtypes
import numpy as np

import concourse.bacc as bacc
import concourse.tile as tile
from concourse import mybir
from concourse.bass_utils import run_bass_kernel_spmd

N1, N2, D, B = 4096, 8192, 128, 1024  # emb1 rows, emb2 rows, dim, num_bits
G1, G2 = 2, 4
M1, M2 = N1 // G1, N2 // G2  # 2048 x 2048 output block per core
KC = B // 128  # 8 bit-chunks of 128
RW = 512  # projection row-chunk width
NW = 512  # main matmul output tile width

_BUILD_CACHE = {}


def _dedupe_ldweights(nc):
    """Drop back-to-back InstLdweights with identical operands on the PE
    queue. The pipeline emits one weight load per matmul; when consecutive
    matmuls share a stationary operand, the reload is pure overhead. Only
    loads carrying no semaphore waits/updates are removed, so sync
    arithmetic is unchanged; the paired matmuls then use the weights the
    earlier identical load left in the array."""
    removed = 0
    for f in nc.m.functions:
        for bb in f.blocks:
            last_key = None
            for ins in list(bb.instructions):
                if type(ins).__name__ == "InstLdweights":
                    key = ins.concise()
                    if (
                        key == last_key
                        and not ins.has_wait()
                        and not ins.has_update()
                    ):
                        bb.instructions.remove(ins)
                        removed += 1
                    else:
                        last_key = key
    return removed


def _build(scale: float):
    if scale in _BUILD_CACHE:
        return _BUILD_CACHE[scale]
    nc = bacc.Bacc("TRN2", target_bir_lowering=False, debug=False)
    f32 = mybir.dt.float32
    bf16 = mybir.dt.bfloat16
    fp8 = mybir.dt.float8e4

    e1h = nc.declare_dram_parameter("e1h", [D, M1], bf16, isOutput=False)
    e1l = nc.declare_dram_parameter("e1l", [D, M1], bf16, isOutput=False)
    e2h = nc.declare_dram_parameter("e2h", [D, M2], bf16, isOutput=False)
    e2l = nc.declare_dram_parameter("e2l", [D, M2], bf16, isOutput=False)
    rh = nc.declare_dram_parameter("rh", [D, B], bf16, isOutput=False)
    rl = nc.declare_dram_parameter("rl", [D, B], bf16, isOutput=False)
    out = nc.declare_dram_parameter("out", [M1, M2], f32, isOutput=True)

    with tile.TileContext(nc) as tc:
        with (
            tc.tile_pool(name="const", bufs=1) as const_pool,
            tc.tile_pool(name="codes", bufs=1) as code_pool,
            tc.tile_pool(name="outs", bufs=3) as out_pool,
            tc.tile_pool(name="psum", bufs=4, space="PSUM") as psum_pool,
        ):
            # The first projection group needs rh/rl bit-chunks 0-1 and the
            # first e1 row-chunk. Spread the gating loads over three DMA
            # queues (sync: r head, scalar: first e1 chunk, gpsimd: rest)
            # so their issue costs overlap.
            rh_sb = const_pool.tile([D, B], bf16)
            rl_sb = const_pool.tile([D, B], bf16)
            nc.sync.dma_start(rh_sb[:, 0:256], rh[:, 0:256])
            nc.sync.dma_start(rl_sb[:, 0:256], rl[:, 0:256])
            nc.sync.dma_start(rh_sb[:, 256:], rh[:, 256:])
            nc.sync.dma_start(rl_sb[:, 256:], rl[:, 256:])

            e1h_sb = const_pool.tile([D, M1], bf16)
            e1l_sb = const_pool.tile([D, M1], bf16)
            e2h_sb = const_pool.tile([D, M2], bf16)
            e2l_sb = const_pool.tile([D, M2], bf16)

            ut = code_pool.tile([128, KC, M1], fp8)
            vt = code_pool.tile([128, KC, M2], fp8)

            # HAM warm-up: the PE clock sits at 1.2 GHz until ~3.4us of
            # sustained activity. Burn dummy matmuls (on whatever the SBUF
            # tile happens to contain - results are discarded) while the
            # input DMAs are in flight so the projections start at 2.4 GHz.
            warm = const_pool.tile([128, RW], bf16)
            nc.vector.memset(warm[:], 0.0)
            wps = psum_pool.tile([128, 2, RW], f32, name="pstile", tag="ps")
            for w in range(10):
                nc.tensor.matmul(
                    wps[:, w % 2, :], warm[:, 0:128], warm[:],
                    start=(w < 2), stop=(w >= 8),
                )

            # Projection: psum[bits 128, rows 512] accumulates
            # rh.T@eh + rh.T@el + rl.T@eh for two bit-chunks per 2-bank
            # psum tile, then one DVE op makes +/-0.5 fp8 codes with bits
            # on partitions. Row-chunk outer so each chunk's DMA is
            # consumed immediately.
            srcs = (
                (e1h_sb, e1l_sb, e1h, e1l, ut, M1),
                (e2h_sb, e2l_sb, e2h, e2l, vt, M2),
            )
            first = True
            for hsb, lsb, hdr, ldr, dst, rows in srcs:
                for j in range(rows // RW):
                    sl = slice(j * RW, (j + 1) * RW)
                    dma_eng = nc.scalar if first else nc.gpsimd
                    first = False
                    dma_eng.dma_start(hsb[:, sl], hdr[:, sl])
                    dma_eng.dma_start(lsb[:, sl], ldr[:, sl])
                    for c2 in range(KC // 2):
                        ps = psum_pool.tile([128, 2, RW], f32, name="pstile", tag="ps")
                        for h in range(2):
                            cs = slice((2 * c2 + h) * 128, (2 * c2 + h + 1) * 128)
                            nc.tensor.matmul(
                                ps[:, h, :], rh_sb[:, cs], hsb[:, sl],
                                start=True, stop=False,
                            )
                            nc.tensor.matmul(
                                ps[:, h, :], rh_sb[:, cs], lsb[:, sl],
                                start=False, stop=False,
                            )
                            nc.tensor.matmul(
                                ps[:, h, :], rl_sb[:, cs], hsb[:, sl],
                                start=False, stop=True,
                            )
                        nc.vector.tensor_scalar(
                            dst[:, 2 * c2 : 2 * c2 + 2, sl],
                            ps[:],
                            0.0,
                            0.5,
                            mybir.AluOpType.is_gt,
                            mybir.AluOpType.subtract,
                        )

            # Main code matmul: per row-block m, two 2-bank psum tiles
            # (same rotating pool as the projections - the first main
            # groups interleave with the projection tail) cover the full
            # 2048-wide n range; the s-loop is outer so consecutive
            # DoubleRow matmuls share one stationary operand (redundant
            # LDWEIGHTS deduped post-compile), then one Sin per 1024-wide
            # half and a 512 KB output DMA each.
            for m in range(M1 // 128):
                ot = out_pool.tile([128, M2], f32)
                ms = slice(m * 128, (m + 1) * 128)
                pss = [
                    psum_pool.tile([128, 2, NW], f32, name="pstile", tag="ps")
                    for _ in range(2)
                ]
                for s in range(KC // 2):
                    for t in range(2):
                        for h in range(2):
                            n = 2 * t + h
                            ns = slice(n * NW, (n + 1) * NW)
                            nc.tensor.matmul(
                                pss[t][:, h, :],
                                ut[:, 2 * s : 2 * s + 2, ms],
                                vt[:, 2 * s : 2 * s + 2, ns],
                                start=(s == 0),
                                stop=(s == KC // 2 - 1),
                                perf_mode=mybir.MatmulPerfMode.DoubleRow,
                            )
                last_m = m == M1 // 128 - 1
                for t in range(2):
                    os_ = slice(t * 2 * NW, (t + 1) * 2 * NW)
                    nc.scalar.activation(
                        ot[:, os_],
                        pss[t][:],
                        mybir.ActivationFunctionType.Sin,
                        scale=scale,
                    )
                    # alternate output stores across two queues so the exit
                    # drain isn't gated on a single queue's backlog; split
                    # the final row 4-ways so the last transfer is small
                    if last_m:
                        h0 = slice(t * 2 * NW, t * 2 * NW + NW)
                        h1 = slice(t * 2 * NW + NW, (t + 1) * 2 * NW)
                        nc.sync.dma_start(out[ms, h0], ot[:, h0])
                        nc.gpsimd.dma_start(out[ms, h1], ot[:, h1])
                    else:
                        (nc.sync if t == 0 else nc.gpsimd).dma_start(
                            out[ms, os_], ot[:, os_]
                        )

    # Keep waits on the matmuls (not hoisted to ldweights) so redundant
    # weight loads stay sync-free and can be deduped away.
    nc.move_matmul_waits_to_ldweights = lambda: None
    nc.compile()
    _dedupe_ldweights(nc)
    _BUILD_CACHE[scale] = nc
    return nc


def _split(x):
    hi = x.astype(ml_dtypes.bfloat16)
    lo = (x - hi.astype(np.float32)).astype(ml_dtypes.bfloat16)
    return hi, lo


def _in_maps(emb1, emb2, r):
    rh, rl = _split(np.ascontiguousarray(r.T))
    e1h, e1l = _split(np.ascontiguousarray(emb1.T))
    e2h, e2l = _split(np.ascontiguousarray(emb2.T))
    maps = []
    for k in range(8):
        a, b = k // G2, k % G2
        s1 = slice(a * M1, (a + 1) * M1)
        s2 = slice(b * M2, (b + 1) * M2)
        maps.append(
            {
                "e1h": np.ascontiguousarray(e1h[:, s1]),
                "e1l": np.ascontiguousarray(e1l[:, s1]),
                "e2h": np.ascontiguousarray(e2h[:, s2]),
                "e2l": np.ascontiguousarray(e2l[:, s2]),
                "rh": rh,
                "rl": rl,
            }
        )
    return maps


def _install_profile_hook():
    """The agent image's antenv lacks axon_hooks; synthesize it so
    run_bass_kernel_spmd(trace=True) can reach the NTFF profiler."""
    import types

    if "antenv.axon_hooks" in sys.modules:
        return
    try:
        from trn_agent_boot.trn_boot import _ntff_profile_via_ctypes

        hook = _ntff_profile_via_ctypes("/opt/axon/libaxon_pjrt.so")
        mod = types.ModuleType("antenv.axon_hooks")
        mod.get_axon_ntff_profile_hook = lambda: hook
        sys.modules["antenv.axon_hooks"] = mod

        from concourse import bass_utils as _bu

        _orig_upload = _bu.upload_artifacts

        def _safe_upload(tmpdir):
            try:
                return _orig_upload(tmpdir)
            except Exception as e:  # no bucket access in this container
                return f"upload-skipped: {e}"

        _bu.upload_artifacts = _safe_upload
    except Exception:
        pass


def kernel(emb1, emb2, r, pi, _trace=False, _tmpdir=None):
    emb1 = np.asarray(emb1, dtype=np.float32)
    emb2 = np.asarray(emb2, dtype=np.float32)
    r = np.asarray(r, dtype=np.float32)
    # codes are half-signs (+/-0.5): dot = s_u.s_v / 4, so scale is 4x pi/2048
    scale = 4.0 * float(np.asarray(pi).reshape(-1)[0]) / (2.0 * B)

    nc = _build(scale)
    if _trace:
        _install_profile_hook()
    try:
        res = run_bass_kernel_spmd(
            nc, _in_maps(emb1, emb2, r), list(range(8)), trace=_trace, tmpdir=_tmpdir
        )
    except ModuleNotFoundError:
        res = run_bass_kernel_spmd(nc, _in_maps(emb1, emb2, r), list(range(8)))

    full = np.empty((N1, N2), dtype=np.float32)
    for k in range(8):
        a, b = k // G2, k % G2
        full[a * M1 : (a + 1) * M1, b * M2 : (b + 1) * M2] = res.results[k]["out"]
    if _trace:
        kernel._last_exec_time_ns = res.exec_time_ns
    return full


# revision 31
# speedup vs baseline: 1.1946x; 1.1946x over previous
"""LSH cosine-of-Hamming retrieval kernel for 8 trn2 NeuronCores.

Math: reference computes cos((pi/d) * hamming(u, v)) for binary LSH codes
u = (emb1 @ r.T > 0), v = (emb2 @ r.T > 0), d = 1024 bits.
With +/-1 sign codes s_u = 2u-1, s_v = 2v-1:
    hamming = (d - s_u . s_v) / 2
    cos((pi/d) * hamming) = cos(pi/2 - (pi/2d) * s_u.s_v) = sin((pi/2d) * s_u.s_v)
The kernel stores half-codes c = s/2 = (x > 0) - 0.5 in fp8 (exact), so
out = sin((2*pi/d) * c_u.c_v).

Pipeline per core: bf16 hi/lo split projection matmul (3 accumulating
passes hh+hl+lh reproduce fp32 signs to ~1e-6 flip rate) -> one DVE
tensor_scalar (is_gt, subtract 0.5) -> fp8 +/-0.5 codes -> fp8 DoubleRow
code matmul (integer-exact in PSUM f32) -> Sin activation, scale pi/512.

Sharding (2x4 grid over 8 cores): core k computes the [2048, 2048] output
block for emb1 rows [(k//4)*2048...] x emb2 rows [(k%4)*2048...]; r is
replicated. This minimizes replicated projection FLOPs without
cross-core exchange (collectives measure ~60us fixed cost under this
runtime - slower than the 26us of projection work they would save).
Embeddings are passed to the device pre-transposed (dim-major) and
hi/lo-split on the host - pure data layout prep.
"""

import sys

sys.path.insert(0, "/opt/trn_rl_repo")

import ml_dtypes
import numpy as np

import concourse.bacc as bacc
import concourse.tile as tile
from concourse import mybir
from concourse.bass_utils import run_bass_kernel_spmd

N1, N2, D, B = 4096, 8192, 128, 1024  # emb1 rows, emb2 rows, dim, num_bits
G1, G2 = 2, 4
M1, M2 = N1 // G1, N2 // G2  # 2048 x 2048 output block per core
KC = B // 128  # 8 bit-chunks of 128
RW = 512  # projection row-chunk width
NW = 512  # main matmul output tile width

_BUILD_CACHE = {}


def _dedupe_ldweights(nc):
    """Drop back-to-back InstLdweights with identical operands on the PE
    queue. The pipeline emits one weight load per matmul; when consecutive
    matmuls share a stationary operand, the reload is pure overhead. Only
    loads carrying no semaphore waits/updates are removed, so sync
    arithmetic is unchanged; the paired matmuls then use the weights the
    earlier identical load left in the array."""
    removed = 0
    for f in nc.m.functions:
        for bb in f.blocks:
            last_key = None
            for ins in list(bb.instructions):
                if type(ins).__name__ == "InstLdweights":
                    key = ins.concise()
                    if (
                        key == last_key
                        and not ins.has_wait()
                        and not ins.has_update()
                    ):
                        bb.instructions.remove(ins)
                        removed += 1
                    else:
                        last_key = key
    return removed


def _build(scale: float):
    if scale in _BUILD_CACHE:
        return _BUILD_CACHE[scale]
    nc = bacc.Bacc("TRN2", target_bir_lowering=False, debug=False)
    f32 = mybir.dt.float32
    bf16 = mybir.dt.bfloat16
    fp8 = mybir.dt.float8e4

    e1h = nc.declare_dram_parameter("e1h", [D, M1], bf16, isOutput=False)
    e1l = nc.declare_dram_parameter("e1l", [D, M1], bf16, isOutput=False)
    e2h = nc.declare_dram_parameter("e2h", [D, M2], bf16, isOutput=False)
    e2l = nc.declare_dram_parameter("e2l", [D, M2], bf16, isOutput=False)
    rh = nc.declare_dram_parameter("rh", [D, B], bf16, isOutput=False)
    rl = nc.declare_dram_parameter("rl", [D, B], bf16, isOutput=False)
    out = nc.declare_dram_parameter("out", [M1, M2], f32, isOutput=True)

    with tile.TileContext(nc) as tc:
        with (
            tc.tile_pool(name="const", bufs=1) as const_pool,
            tc.tile_pool(name="codes", bufs=1) as code_pool,
            tc.tile_pool(name="outs", bufs=3) as out_pool,
            tc.tile_pool(name="psum", bufs=4, space="PSUM") as psum_pool,
        ):
            # The first projection group needs rh/rl bit-chunks 0-1 and the
            # first e1 row-chunk. Spread the gating loads over three DMA
            # queues (sync: r head, scalar: first e1 chunk, gpsimd: rest)
            # so their issue costs overlap.
            rh_sb = const_pool.tile([D, B], bf16)
            rl_sb = const_pool.tile([D, B], bf16)
            nc.sync.dma_start(rh_sb[:, 0:256], rh[:, 0:256])
            nc.sync.dma_start(rl_sb[:, 0:256], rl[:, 0:256])
            nc.sync.dma_start(rh_sb[:, 256:], rh[:, 256:])
            nc.sync.dma_start(rl_sb[:, 256:], rl[:, 256:])

            e1h_sb = const_pool.tile([D, M1], bf16)
            e1l_sb = const_pool.tile([D, M1], bf16)
            e2h_sb = const_pool.tile([D, M2], bf16)
            e2l_sb = const_pool.tile([D, M2], bf16)

            ut = code_pool.tile([128, KC, M1], fp8)
            vt = code_pool.tile([128, KC, M2], fp8)

            # HAM warm-up: the PE clock sits at 1.2 GHz until ~3.4us of
            # sustained activity. Burn dummy matmuls (on whatever the SBUF
            # tile happens to contain - results are discarded) while the
            # input DMAs are in flight so the projections start at 2.4 GHz.
            warm = const_pool.tile([128, RW], bf16)
            nc.vector.memset(warm[:], 0.0)
            wps = psum_pool.tile([128, 2, RW], f32, name="pstile", tag="ps")
            for w in range(10):
                nc.tensor.matmul(
                    wps[:, w % 2, :], warm[:, 0:128], warm[:],
                    start=(w < 2), stop=(w >= 8),
                )

            # Projection: psum[bits 128, rows 512] accumulates
            # rh.T@eh + rh.T@el + rl.T@eh for two bit-chunks per 2-bank
            # psum tile, then one DVE op makes +/-0.5 fp8 codes with bits
            # on partitions. Row-chunk outer so each chunk's DMA is
            # consumed immediately.
            srcs = (
                (e1h_sb, e1l_sb, e1h, e1l, ut, M1),
                (e2h_sb, e2l_sb, e2h, e2l, vt, M2),
            )
            first = True
            for hsb, lsb, hdr, ldr, dst, rows in srcs:
                for j in range(rows // RW):
                    sl = slice(j * RW, (j + 1) * RW)
                    dma_eng = nc.scalar if first else nc.gpsimd
                    first = False
                    dma_eng.dma_start(hsb[:, sl], hdr[:, sl])
                    dma_eng.dma_start(lsb[:, sl], ldr[:, sl])
                    for c2 in range(KC // 2):
                        ps = psum_pool.tile([128, 2, RW], f32, name="pstile", tag="ps")
                        for h in range(2):
                            cs = slice((2 * c2 + h) * 128, (2 * c2 + h + 1) * 128)
                            nc.tensor.matmul(
                                ps[:, h, :], rh_sb[:, cs], hsb[:, sl],
                                start=True, stop=False,
                            )
                            nc.tensor.matmul(
                                ps[:, h, :], rh_sb[:, cs], lsb[:, sl],
                                start=False, stop=False,
                            )
                            nc.tensor.matmul(
                                ps[:, h, :], rl_sb[:, cs], hsb[:, sl],
                                start=False, stop=True,
                            )
                        nc.vector.tensor_scalar(
                            dst[:, 2 * c2 : 2 * c2 + 2, sl],
                            ps[:],
                            0.0,
                            0.5,
                            mybir.AluOpType.is_gt,
                            mybir.AluOpType.subtract,
                        )

            # Main code matmul: per row-block m, two 2-bank psum tiles
            # (same rotating pool as the projections - the first main
            # groups interleave with the projection tail) cover the full
            # 2048-wide n range; the s-loop is outer so consecutive
            # DoubleRow matmuls share one stationary operand (redundant
            # LDWEIGHTS deduped post-compile), then one Sin per 1024-wide
            # half and a 512 KB output DMA each.
            for m in range(M1 // 128):
                ot = out_pool.tile([128, M2], f32)
                ms = slice(m * 128, (m + 1) * 128)
                pss = [
                    psum_pool.tile([128, 2, NW], f32, name="pstile", tag="ps")
                    for _ in range(2)
                ]
                for s in range(KC // 2):
                    for t in range(2):
                        for h in range(2):
                            n = 2 * t + h
                            ns = slice(n * NW, (n + 1) * NW)
                            nc.tensor.matmul(
                                pss[t][:, h, :],
                                ut[:, 2 * s : 2 * s + 2, ms],
                                vt[:, 2 * s : 2 * s + 2, ns],
                                start=(s == 0),
                                stop=(s == KC // 2 - 1),
                                perf_mode=mybir.MatmulPerfMode.DoubleRow,
                            )
                for t in range(2):
                    os_ = slice(t * 2 * NW, (t + 1) * 2 * NW)
                    nc.scalar.activation(
                        ot[:, os_],
                        pss[t][:],
                        mybir.ActivationFunctionType.Sin,
                        scale=scale,
                    )
                    # alternate output stores across two queues so the exit
                    # drain isn't gated on a single queue's backlog
                    (nc.sync if t == 0 else nc.gpsimd).dma_start(
                        out[ms, os_], ot[:, os_]
                    )

    # Keep waits on the matmuls (not hoisted to ldweights) so redundant
    # weight loads stay sync-free and can be deduped away.
    nc.move_matmul_waits_to_ldweights = lambda: None
    nc.compile()
    _dedupe_ldweights(nc)
    _BUILD_CACHE[scale] = nc
    return nc


def _split(x):
    hi = x.astype(ml_dtypes.bfloat16)
    lo = (x - hi.astype(np.float32)).astype(ml_dtypes.bfloat16)
    return hi, lo


def _in_maps(emb1, emb2, r):
    rh, rl = _split(np.ascontiguousarray(r.T))
    e1h, e1l = _split(np.ascontiguousarray(emb1.T))
    e2h, e2l = _split(np.ascontiguousarray(emb2.T))
    maps = []
    for k in range(8):
        a, b = k // G2, k % G2
        s1 = slice(a * M1, (a + 1) * M1)
        s2 = slice(b * M2, (b + 1) * M2)
        maps.append(
            {
                "e1h": np.ascontiguousarray(e1h[:, s1]),
                "e1l": np.ascontiguousarray(e1l[:, s1]),
                "e2h": np.ascontiguousarray(e2h[:, s2]),
                "e2l": np.ascontiguousarray(e2l[:, s2]),
                "rh": rh,
                "rl": rl,
            }
        )
    return maps


def _install_profile_hook():
    """The agent image's antenv lacks axon_hooks; synthesize it so
    run_bass_kernel_spmd(trace=True) can reach the NTFF profiler."""
    import types

    if "antenv.axon_hooks" in sys.modules:
        return
    try:
        from trn_agent_boot.trn_boot import _ntff_profile_via_ctypes

        hook = _ntff_profile_via_ctypes("/opt/axon/libaxon_pjrt.so")
        mod = types.ModuleType("antenv.axon_hooks")
        mod.get_axon_ntff_profile_hook = lambda: hook
        sys.modules["antenv.axon_hooks"] = mod

        from concourse import bass_utils as _bu

        _orig_upload = _bu.upload_artifacts

        def _safe_upload(tmpdir):
            try:
                return _orig_upload(tmpdir)
            except Exception as e:  # no bucket access in this container
                return f"upload-skipped: {e}"

        _bu.upload_artifacts = _safe_upload
    except Exception:
        pass


def kernel(emb1, emb2, r, pi, _trace=False, _tmpdir=None):
    emb1 = np.asarray(emb1, dtype=np.float32)
    emb2 = np.asarray(emb2, dtype=np.float32)
    r = np.asarray(r, dtype=np.float32)
    # codes are half-signs (+/-0.5): dot = s_u.s_v / 4, so scale is 4x pi/2048
    scale = 4.0 * float(np.asarray(pi).reshape(-1)[0]) / (2.0 * B)

    nc = _build(scale)
    if _trace:
        _install_profile_hook()
    try:
        res = run_bass_kernel_spmd(
            nc, _in_maps(emb1, emb2, r), list(range(8)), trace=_trace, tmpdir=_tmpdir
        )
    except ModuleNotFoundError:
        res = run_bass_kernel_spmd(nc, _in_maps(emb1, emb2, r), list(range(8)))

    full = np.empty((N1, N2), dtype=np.float32)
    for k in range(8):
        a, b = k // G2, k % G2
        full[a * M1 : (a + 1) * M1, b * M2 : (b + 1) * M2] = res.results[k]["out"]
    if _trace:
        kernel._last_exec_time_ns = res.exec_time_ns
    return full
